# revision 10
# baseline (speedup 1.0000x reference)
"""Trainium2 Bass kernel for nn_LATTICE (multimodal GNN, 8-core SPMD).

Takes FULL inputs (as produced by setup_inputs()), shards across 8 NeuronCores,
runs one Bass kernel, and reassembles the full output (u_g, i_g + h_norm).
"""

import os
import sys

sys.path.insert(0, "/opt/trn_rl_repo")

import numpy as np

import concourse.bacc as bacc
import concourse.mybir as mybir
import concourse.tile as tile
from concourse import bass
from concourse.bass_utils import run_bass_kernel_spmd
from concourse.masks import make_identity

# ---------------------------------------------------------------- constants
N_USERS = 8192
N_ITEMS = 4096
D = 64
IMG_D = 4096
TXT_D = 384
TOPK = 10
LAMBDA = 0.9
NCORES = 8
M = N_USERS + N_ITEMS

R = N_ITEMS // NCORES            # 512 item rows per core
UPC = N_USERS // NCORES          # 1024 user rows per core
LROWS = UPC + R                  # 1536 lightgcn rows per core
NWIN = LROWS // 128              # 12 windows of 128 output rows
NRT = R // 128                   # 4 item row-tiles per core

FP = mybir.dt.float32
I32 = mybir.dt.int32
AF = mybir.ActivationFunctionType
OP = mybir.AluOpType

NEG_BIG = -1.0e30

# last run results, for test harness introspection
LAST_RESULTS = None

_BUILD_CACHE = {}


def _install_ntff_hook_shim():
    """run_bass_kernel_spmd(trace=True) imports antenv.axon_hooks, which is
    missing in this container. Provide it, backed by trn_boot's ctypes hook."""
    import types

    if "antenv.axon_hooks" in sys.modules:
        return
    mod = types.ModuleType("antenv.axon_hooks")
    state = {"hook": None}

    def set_axon_ntff_profile_hook(h):
        state["hook"] = h

    def get_axon_ntff_profile_hook():
        return state["hook"]

    mod.set_axon_ntff_profile_hook = set_axon_ntff_profile_hook
    mod.get_axon_ntff_profile_hook = get_axon_ntff_profile_hook
    sys.modules["antenv.axon_hooks"] = mod
    try:
        import antenv

        antenv.axon_hooks = mod
    except Exception:
        pass
    try:
        from trn_agent_boot.trn_boot import _ntff_profile_via_ctypes

        hook = _ntff_profile_via_ctypes("/opt/axon/libaxon_pjrt.so")
        if hook is not None:
            set_axon_ntff_profile_hook(hook)
    except Exception:
        pass


_install_ntff_hook_shim()


# ---------------------------------------------------------------- host prep
def _softmax2_f32(x):
    x = np.asarray(x, np.float32)
    e = np.exp(x - x.max())
    w = e / e.sum()
    return float(w[0]), float(w[1])


def _prep_lightgcn(adj_rows, adj_cols, adj_vals, user_emb, item_emb):
    """Sort COO by (owner core, 128-row window), pad windows to a uniform
    per-window tile count T[w] shared by all cores, and build per-core arrays.
    """
    rows = adj_rows.astype(np.int64)
    cols = adj_cols.astype(np.int64)
    vals = adj_vals.astype(np.float32)

    is_user = rows < N_USERS
    owner = np.where(is_user, rows // UPC, (rows - N_USERS) // R)
    local = np.where(is_user, rows - owner * UPC, UPC + (rows - N_USERS - owner * R))
    win = local // 128
    rel = local - win * 128

    # global position of row m inside the AllGather-concatenated ego table
    def g_of(m):
        m = np.asarray(m, np.int64)
        iu = m < N_USERS
        ow = np.where(iu, m // UPC, (m - N_USERS) // R)
        lc = np.where(iu, m - ow * UPC, UPC + (m - N_USERS - ow * R))
        return ow * LROWS + lc

    ego0_full = np.concatenate(
        [user_emb.astype(np.float32), item_emb.astype(np.float32)], axis=0
    )

    # per (core, window) entry lists, sorted by col for gather locality
    counts = np.zeros((NCORES, NWIN), np.int64)
    order = np.lexsort((cols, win, owner))
    rows_s, owner_s, win_s, rel_s, col_s, val_s = (
        rows[order], owner[order], win[order], rel[order], cols[order], vals[order],
    )
    for c in range(NCORES):
        for w in range(NWIN):
            counts[c, w] = np.sum((owner_s == c) & (win_s == w))
    T = np.maximum(1, (counts.max(axis=0) + 127) // 128).astype(np.int64)  # [NWIN]
    Ttot = int(T.sum())
    S = Ttot * 128

    per_core = []
    for c in range(NCORES):
        rel_pad = np.zeros(S, np.float32)
        val_pad = np.zeros(S, np.float32)
        col_pad = np.zeros(S, np.int64)
        base = 0
        for w in range(NWIN):
            sel = (owner_s == c) & (win_s == w)
            k = int(counts[c, w])
            cap = int(T[w]) * 128
            rel_pad[base : base + k] = rel_s[sel].astype(np.float32)
            val_pad[base : base + k] = val_s[sel]
            col_pad[base : base + k] = col_s[sel]
            base += cap
        g0 = ego0_full[col_pad]                       # [S, D] layer-1 pre-gather
        l2c = g_of(col_pad).astype(np.int32)          # [S] indices into AG'd ego1
        # [128, Ttot] layouts: column t, partition p -> slot t*128+p
        rel2d = np.ascontiguousarray(rel_pad.reshape(Ttot, 128).T)
        val2d = np.ascontiguousarray(val_pad.reshape(Ttot, 128).T)
        l2c2d = np.ascontiguousarray(l2c.reshape(Ttot, 128).T)
        per_core.append(
            dict(g0=np.ascontiguousarray(g0), rel2d=rel2d, val2d=val2d, l2c2d=l2c2d)
        )
    return tuple(int(t) for t in T), per_core


# ---------------------------------------------------------------- bass build
def _build(Twins, w0, w1):
    """Build the 8-core SPMD bass program. Twins: per-window tile counts."""
    Ttot = int(sum(Twins))
    S = Ttot * 128
    NK_IMG = IMG_D // 128   # 32
    NK_TXT = TXT_D // 128   # 3
    NCH = N_ITEMS // 128    # 32 column chunks of the item graph

    nc = bacc.Bacc("TRN2", target_bir_lowering=False, debug=False,
                   num_devices=NCORES)

    # ---------------- I/O
    featsT_img = nc.dram_tensor("featsT_img", [IMG_D, R], FP, kind="ExternalInput")
    featsT_txt = nc.dram_tensor("featsT_txt", [TXT_D, R], FP, kind="ExternalInput")
    w_img = nc.dram_tensor("w_img", [IMG_D, D], FP, kind="ExternalInput")
    w_txt = nc.dram_tensor("w_txt", [TXT_D, D], FP, kind="ExternalInput")
    b_img = nc.dram_tensor("b_img", [D, 1], FP, kind="ExternalInput")
    b_txt = nc.dram_tensor("b_txt", [D, 1], FP, kind="ExternalInput")
    origT_img = nc.dram_tensor("origT_img", [N_ITEMS, R], FP, kind="ExternalInput")
    origT_txt = nc.dram_tensor("origT_txt", [N_ITEMS, R], FP, kind="ExternalInput")
    item_emb = nc.dram_tensor("item_emb", [N_ITEMS, D], FP, kind="ExternalInput")
    ego0_loc = nc.dram_tensor("ego0_loc", [LROWS, D], FP, kind="ExternalInput")
    g0_in = nc.dram_tensor("g0_in", [S, D], FP, kind="ExternalInput")
    rel_in = nc.dram_tensor("rel_in", [128, Ttot], FP, kind="ExternalInput")
    val_in = nc.dram_tensor("val_in", [128, Ttot], FP, kind="ExternalInput")
    l2c_in = nc.dram_tensor("l2c_in", [128, Ttot], I32, kind="ExternalInput")

    u_out = nc.dram_tensor("u_out", [UPC, D], FP, kind="ExternalOutput")
    i_out = nc.dram_tensor("i_out", [R, D], FP, kind="ExternalOutput")
    dbg_wa = nc.dram_tensor("dbg_wa", [128, N_ITEMS], FP, kind="ExternalOutput")
    dbg_h = nc.dram_tensor("dbg_h", [128, D], FP, kind="ExternalOutput")
    dbg_rs = nc.dram_tensor("dbg_rs", [128, 1], FP, kind="ExternalOutput")

    rg = [list(range(NCORES))]

    with tile.TileContext(nc) as tc:
        with (
            tc.tile_pool(name="pers", bufs=1) as pers,          # persistent sbuf
            tc.tile_pool(name="stream", bufs=3) as stream,      # streaming loads
            tc.tile_pool(name="work", bufs=1) as work,          # big work buffers
            tc.tile_pool(name="small", bufs=2) as small,        # small scratch
            tc.tile_pool(name="dram", bufs=1, space="DRAM") as dram,
            tc.tile_pool(name="pA", bufs=2, space="PSUM") as pA,
            tc.tile_pool(name="pSim", bufs=1, space="PSUM") as pSim,
            tc.tile_pool(name="pTr", bufs=1, space="PSUM") as pTr,
            tc.tile_pool(name="pH", bufs=1, space="PSUM") as pH,
            tc.tile_pool(name="pE", bufs=2, space="PSUM") as pE,
        ):
            # ---------------- constants
            identity = pers.tile([128, 128], FP)
            make_identity(nc, identity[:])
            iota_i = pers.tile([128, 128], I32)
            nc.gpsimd.iota(iota_i[:], pattern=[[1, 128]], base=0, channel_multiplier=0)
            iota_f = pers.tile([128, 128], FP)
            nc.vector.tensor_copy(iota_f[:], iota_i[:])
            ones_k = pers.tile([64, 64], FP)
            nc.vector.memset(ones_k[:], 1.0)

            # ---------------- collective bounce buffers
            ag_xn_in = dram.tile([128, R], FP)
            ag_xn_out = dram.tile([NCORES * 128, R], FP, addr_space="Shared")
            ag_rs_in = dram.tile([R, 1], FP)
            ag_rs_out = dram.tile([N_ITEMS, 1], FP, addr_space="Shared")
            ag_e1_in = dram.tile([LROWS, D], FP)
            ag_e1_out = dram.tile([NCORES * LROWS, D], FP, addr_space="Shared")

            # ================= Phase A: modal transforms (transposed layout)
            xnTl = [pers.tile([64, R], FP, name=f"xnTl{m}", tag=f"xnTl{m}")
                    for m in range(2)]
            for mod, (K, NK, ftens, wtens, btens, prow) in enumerate(
                [
                    (IMG_D, NK_IMG, featsT_img, w_img, b_img, 0),
                    (TXT_D, NK_TXT, featsT_txt, w_txt, b_txt, 64),
                ]
            ):
                mm = pA.tile([64, R], FP, name=f"pa_mm{mod}", tag="pa_mm")
                for k in range(NK):
                    wt = stream.tile([128, D], FP, name=f"wt{mod}_{k}", tag="wt")
                    nc.sync.dma_start(out=wt[:], in_=wtens[k * 128 : (k + 1) * 128, :])
                    ft = stream.tile([128, R], FP, name=f"ft{mod}_{k}", tag="ft")
                    nc.sync.dma_start(out=ft[:], in_=ftens[k * 128 : (k + 1) * 128, :])
                    nc.tensor.matmul(mm[:], lhsT=wt[:], rhs=ft[:],
                                     start=(k == 0), stop=(k == NK - 1))
                bias = small.tile([64, 1], FP, name=f"bias{mod}", tag="bias")
                nc.sync.dma_start(out=bias[:], in_=btens[:, :])
                mT = work.tile([64, R], FP, name=f"mT{mod}", tag="mT")
                nc.vector.tensor_scalar(mT[:], mm[:], bias[:], None, op0=OP.add)
                sq = work.tile([64, R], FP, name=f"sq{mod}", tag="sq")
                nc.vector.tensor_tensor(sq[:], mT[:], mT[:], op=OP.mult)
                n2 = pA.tile([64, R], FP, name=f"pa_n2{mod}", tag="pa_mm")
                nc.tensor.matmul(n2[:1, :], lhsT=ones_k[:, :1], rhs=sq[:],
                                 start=True, stop=True)
                nrm = small.tile([1, R], FP, name=f"nrm{mod}", tag="nrm")
                nc.scalar.activation(nrm[:], n2[:1, :], AF.Sqrt)
                inv = small.tile([1, R], FP, name=f"inv{mod}", tag="inv")
                nc.vector.reciprocal(inv[:], nrm[:])
                bc = pA.tile([64, R], FP, name=f"pa_bc{mod}", tag="pa_mm")
                nc.tensor.matmul(bc[:], lhsT=ones_k[:1, :], rhs=inv[:],
                                 start=True, stop=True)
                nc.vector.tensor_tensor(xnTl[mod][:], mT[:], bc[:], op=OP.mult)

            nc.sync.dma_start(out=ag_xn_in[0:64, :], in_=xnTl[0][:])
            nc.sync.dma_start(out=ag_xn_in[64:128, :], in_=xnTl[1][:])
            nc.gpsimd.collective_compute(
                "AllGather", OP.bypass, replica_groups=rg,
                ins=[ag_xn_in.opt()], outs=[ag_xn_out.opt()],
            )
            xnT_all = pers.tile([64, N_ITEMS], FP)
            agv = ag_xn_out.rearrange("(r p) c -> p r c", r=NCORES, p=128)

            # ================= Phase D: lightgcn layer 1 (independent of A-C)
            rel_sb = pers.tile([128, Ttot], FP)
            nc.sync.dma_start(out=rel_sb[:], in_=rel_in[:, :])
            val_sb = pers.tile([128, Ttot], FP)
            nc.sync.dma_start(out=val_sb[:], in_=val_in[:, :])
            l2c_sb = pers.tile([128, Ttot], I32)
            nc.sync.dma_start(out=l2c_sb[:], in_=l2c_in[:, :])
            ego0_sb = pers.tile([128, NWIN * D], FP)
            nc.sync.dma_start(
                out=ego0_sb[:].rearrange("p (w d) -> p w d", w=NWIN),
                in_=ego0_loc[:, :].rearrange("(w p) d -> p w d", p=128),
            )
            ego1_sb = pers.tile([128, NWIN * D], FP)
            ego2_sb = pers.tile([128, NWIN * D], FP)

            def spmm_layer(layer, out_sb):
                tbase = 0
                for w in range(NWIN):
                    Tw = Twins[w]
                    acc = pE.tile([128, D], FP, name=f"eacc{layer}_{w}", tag="eacc")
                    for t in range(Tw):
                        j = tbase + t
                        if layer == 1:
                            gt = stream.tile([128, D], FP, name=f"g1_{w}_{t}", tag="gt")
                            nc.sync.dma_start(
                                out=gt[:], in_=g0_in[j * 128 : (j + 1) * 128, :]
                            )
                        else:
                            gt = stream.tile([128, D], FP, name=f"g2_{w}_{t}", tag="gt")
                            nc.gpsimd.indirect_dma_start(
                                out=gt[:],
                                out_offset=None,
                                in_=ag_e1_out[:],
                                in_offset=bass.IndirectOffsetOnAxis(
                                    ap=l2c_sb[:, j : j + 1], axis=0
                                ),
                            )
                        oh = stream.tile([128, 128], FP, name=f"oh{layer}_{w}_{t}",
                                         tag="oh", bufs=4)
                        nc.vector.tensor_scalar(
                            oh[:], iota_f[:], rel_sb[:, j : j + 1],
                            val_sb[:, j : j + 1], op0=OP.is_equal, op1=OP.mult,
                        )
                        nc.tensor.matmul(acc[:], lhsT=oh[:], rhs=gt[:],
                                         start=(t == 0), stop=(t == Tw - 1))
                    nc.scalar.activation(
                        out_sb[:, w * D : (w + 1) * D], acc[:], AF.Copy
                    )
                    tbase += Tw

            spmm_layer(1, ego1_sb)
            nc.sync.dma_start(
                out=ag_e1_in.rearrange("(w p) d -> p w d", p=128),
                in_=ego1_sb[:].rearrange("p (w d) -> p w d", w=NWIN),
            )
            nc.gpsimd.collective_compute(
                "AllGather", OP.bypass, replica_groups=rg,
                ins=[ag_e1_in.opt()], outs=[ag_e1_out.opt()],
            )

            # ================= Phase B: sim + top-k + masked wa
            wa = [work.tile([128, N_ITEMS], FP, name=f"wa{i}", tag=f"wa{i}")
                  for i in range(NRT)]
            rs_loc = [small.tile([128, 1], FP, name=f"rs{i}", tag=f"rs{i}")
                      for i in range(NRT)]
            zap = work.tile([128, N_ITEMS], FP, name="zap", tag="zap")
            wsim = work.tile([128, N_ITEMS], FP, name="wsim", tag="wsim")
            for mod, wmod in [(0, w0), (1, w1)]:
                nc.sync.dma_start(
                    out=xnT_all[:].rearrange("p (r c) -> p r c", r=NCORES),
                    in_=agv[mod * 64 : mod * 64 + 64],
                )
                for i in range(NRT):
                    lhs = xnTl[mod][:, i * 128 : (i + 1) * 128]
                    for half in range(4):
                        ps = pSim.tile([128, 1024], FP, name=f"psim{i}_{mod}_{half}",
                                       tag="psim")
                        for q in range(2):
                            cl = half * 1024 + q * 512
                            nc.tensor.matmul(
                                ps[:, q * 512 : (q + 1) * 512],
                                lhsT=lhs,
                                rhs=xnT_all[:, cl : cl + 512],
                                start=True, stop=True,
                            )
                        nc.scalar.activation(
                            wsim[:, half * 1024 : (half + 1) * 1024], ps[:],
                            AF.Copy, scale=float(wmod),
                        )
                    r1 = small.tile([128, 8], FP, name=f"r1_{i}_{mod}", tag="r8")
                    nc.vector.max(out=r1[:], in_=wsim[:])
                    nc.vector.match_replace(
                        out=zap[:], in_to_replace=r1[:], in_values=wsim[:],
                        imm_value=NEG_BIG,
                    )
                    r2 = small.tile([128, 8], FP, name=f"r2_{i}_{mod}", tag="r8")
                    nc.vector.max(out=r2[:], in_=zap[:])
                    # masked+weighted values for this modality
                    dst = wa[i] if mod == 0 else zap
                    acc_rs = small.tile([128, 1], FP, name=f"accrs{i}_{mod}",
                                        tag="accrs")
                    nc.vector.scalar_tensor_tensor(
                        out=dst[:], in0=wsim[:], scalar=r2[:, 1:2], in1=wsim[:],
                        op0=OP.is_ge, op1=OP.mult, accum_out=acc_rs[:],
                    )
                    if mod == 0:
                        nc.vector.tensor_copy(rs_loc[i][:], acc_rs[:])
                    else:
                        nc.vector.tensor_add(rs_loc[i][:], rs_loc[i][:], acc_rs[:])
                        nc.vector.tensor_add(wa[i][:], wa[i][:], zap[:])
            for i in range(NRT):
                nc.sync.dma_start(
                    out=ag_rs_in[i * 128 : (i + 1) * 128, :], in_=rs_loc[i][:]
                )
            nc.gpsimd.collective_compute(
                "AllGather", OP.bypass, replica_groups=rg,
                ins=[ag_rs_in.opt()], outs=[ag_rs_out.opt()],
            )

            # ================= Phase C: h = 0.1*lap@emb + 0.9*orig@emb, normalize
            # d vector for all 4096 columns
            rs_all = pers.tile([128, NCH], FP)
            nc.sync.dma_start(
                out=rs_all[:].rearrange("p (k o) -> p k o", k=NCH),
                in_=ag_rs_out[:, :].rearrange("(k p) o -> p k o", p=128),
            )
            srt = pers.tile([128, NCH], FP)
            nc.scalar.activation(srt[:], rs_all[:], AF.Sqrt)
            d_all = pers.tile([128, NCH], FP)
            nc.vector.reciprocal(d_all[:], srt[:])

            emb_sb = pers.tile([128, NCH * D], FP)
            nc.sync.dma_start(
                out=emb_sb[:].rearrange("p (k d) -> p k d", k=NCH),
                in_=item_emb[:, :].rearrange("(k p) d -> p k d", p=128),
            )
            demb = pers.tile([128, NCH * D], FP)
            for k in range(NCH):
                nc.vector.tensor_scalar(
                    demb[:, k * D : (k + 1) * D], emb_sb[:, k * D : (k + 1) * D],
                    d_all[:, k : k + 1], (1.0 - LAMBDA),
                    op0=OP.mult, op1=OP.mult,
                )

            # own-row d: from local rowsums
            d_own = [small.tile([128, 1], FP, name=f"down{i}", tag=f"down{i}")
                     for i in range(NRT)]
            for i in range(NRT):
                s_i = small.tile([128, 1], FP, name=f"srtl{i}", tag="srtl")
                nc.scalar.activation(s_i[:], rs_loc[i][:], AF.Sqrt)
                nc.vector.reciprocal(d_own[i][:], s_i[:])
                # scale wa rows by d_own (laplacian row scale)
                nc.vector.tensor_scalar_mul(wa[i][:], wa[i][:], d_own[i][:])

            hsb = [small.tile([128, D], FP, name=f"h{i}", tag=f"h{i}")
                   for i in range(NRT)]
            hps = pH.tile([128, NRT * D], FP)
            nc.vector.memset(hps[:], 0.0)
            # 0.9 * (w0*origI + w1*origT) @ emb — k-outer so each orig chunk
            # is loaded once; PSUM groups interleave across the 4 i-regions.
            nc.vector.tensor_scalar_mul(emb_sb[:], emb_sb[:], LAMBDA * w0)
            for mod, otens in enumerate([origT_img, origT_txt]):
                if mod == 1:
                    nc.vector.tensor_scalar_mul(emb_sb[:], emb_sb[:], w1 / w0)
                for k in range(NCH):
                    ot = stream.tile([128, R], FP, name=f"ot{mod}_{k}", tag="ot")
                    nc.sync.dma_start(
                        out=ot[:], in_=otens[k * 128 : (k + 1) * 128, :]
                    )
                    for i in range(NRT):
                        nc.tensor.matmul(
                            hps[:, i * D : (i + 1) * D],
                            lhsT=ot[:, i * 128 : (i + 1) * 128],
                            rhs=emb_sb[:, k * D : (k + 1) * D],
                            start=False, stop=False,
                            skip_group_check=True,
                        )
            # + d_r*wa @ (0.1*d*emb): transpose wa tile blocks then matmul
            for i in range(NRT):
                for kb in range(NCH // 4):
                    tr = pTr.tile([128, 512], FP, name=f"tr{i}_{kb}", tag="tr")
                    for j in range(4):
                        k = kb * 4 + j
                        nc.tensor.transpose(
                            out=tr[:, j * 128 : (j + 1) * 128],
                            in_=wa[i][:, k * 128 : (k + 1) * 128],
                            identity=identity[:],
                        )
                    waT = stream.tile([128, 512], FP, name=f"waT{i}_{kb}",
                                      tag="waT", bufs=2)
                    nc.scalar.activation(waT[:], tr[:], AF.Copy)
                    for j in range(4):
                        k = kb * 4 + j
                        last = kb == NCH // 4 - 1 and j == 3
                        nc.tensor.matmul(
                            hps[:, i * D : (i + 1) * D],
                            lhsT=waT[:, j * 128 : (j + 1) * 128],
                            rhs=demb[:, k * D : (k + 1) * D],
                            start=False, stop=last,
                            skip_group_check=True,
                        )
                nc.vector.tensor_copy(hsb[i][:], hps[:, i * D : (i + 1) * D])
                if i == 0:
                    nc.sync.dma_start(out=dbg_wa[:, :], in_=wa[0][:])
                    nc.sync.dma_start(out=dbg_h[:, :], in_=hsb[0][:])
                    nc.sync.dma_start(out=dbg_rs[:, :], in_=rs_loc[0][:])

            # row-normalize h
            hnorm = pers.tile([128, NRT * D], FP)
            for i in range(NRT):
                hs = small.tile([128, D], FP, name=f"hsq{i}", tag="hsq")
                nc.vector.tensor_tensor(hs[:], hsb[i][:], hsb[i][:], op=OP.mult)
                s = small.tile([128, 1], FP, name=f"hs{i}", tag="hs")
                nc.vector.reduce_sum(s[:], hs[:], axis=mybir.AxisListType.X)
                sr = small.tile([128, 1], FP, name=f"hsr{i}", tag="hsr")
                nc.scalar.activation(sr[:], s[:], AF.Sqrt)
                nc.vector.tensor_scalar_max(sr[:], sr[:], 1.0e-12)
                rinv = small.tile([128, 1], FP, name=f"hrin{i}", tag="hrin")
                nc.vector.reciprocal(rinv[:], sr[:])
                nc.vector.tensor_scalar_mul(
                    hnorm[:, i * D : (i + 1) * D], hsb[i][:], rinv[:]
                )

            # ================= Phase E: lightgcn layer 2
            spmm_layer(2, ego2_sb)

            # ================= Phase F: mean + outputs
            allemb = pers.tile([128, NWIN * D], FP)
            nc.vector.tensor_add(allemb[:], ego0_sb[:], ego1_sb[:])
            nc.vector.tensor_add(allemb[:], allemb[:], ego2_sb[:])
            nc.vector.tensor_scalar_mul(allemb[:], allemb[:], 1.0 / 3.0)
            nc.sync.dma_start(
                out=u_out[:, :].rearrange("(w p) d -> p w d", p=128),
                in_=allemb[:, : 8 * D].rearrange("p (w d) -> p w d", w=8),
            )
            iadd = pers.tile([128, NRT * D], FP)
            nc.vector.tensor_add(iadd[:], allemb[:, 8 * D :], hnorm[:])
            nc.sync.dma_start(
                out=i_out[:, :].rearrange("(w p) d -> p w d", p=128),
                in_=iadd[:].rearrange("p (w d) -> p w d", w=NRT),
            )

    nc.compile()
    return nc


# ---------------------------------------------------------------- entry point
def kernel(user_emb, item_emb, image_feats, text_feats,
           image_trs_w, image_trs_b, text_trs_w, text_trs_b,
           modal_weight, image_orig_adj, text_orig_adj,
           adj_vals, adj_rows, adj_cols):
    global LAST_RESULTS
    f32 = np.float32
    user_emb = np.asarray(user_emb, f32)
    item_emb = np.asarray(item_emb, f32)
    w0, w1 = _softmax2_f32(modal_weight)

    Twins, lg = _prep_lightgcn(adj_rows, adj_cols, adj_vals, user_emb, item_emb)

    key = (Twins, round(w0, 9))
    if key not in _BUILD_CACHE:
        _BUILD_CACHE.clear()
        _BUILD_CACHE[key] = _build(Twins, w0, w1)
    nc = _BUILD_CACHE[key]

    fTi = np.ascontiguousarray(np.asarray(image_feats, f32).T)   # [IMG_D, 4096]
    fTt = np.ascontiguousarray(np.asarray(text_feats, f32).T)    # [TXT_D, 4096]
    oTi = np.ascontiguousarray(np.asarray(image_orig_adj, f32).T)
    oTt = np.ascontiguousarray(np.asarray(text_orig_adj, f32).T)

    in_maps = []
    for c in range(NCORES):
        sl = slice(c * R, (c + 1) * R)
        ego0_loc = np.concatenate(
            [user_emb[c * UPC : (c + 1) * UPC], item_emb[sl]], axis=0
        )
        in_maps.append(
            {
                "featsT_img": np.ascontiguousarray(fTi[:, sl]),
                "featsT_txt": np.ascontiguousarray(fTt[:, sl]),
                "w_img": np.asarray(image_trs_w, f32),
                "w_txt": np.asarray(text_trs_w, f32),
                "b_img": np.asarray(image_trs_b, f32).reshape(D, 1),
                "b_txt": np.asarray(text_trs_b, f32).reshape(D, 1),
                "origT_img": np.ascontiguousarray(oTi[:, sl]),
                "origT_txt": np.ascontiguousarray(oTt[:, sl]),
                "item_emb": item_emb,
                "ego0_loc": np.ascontiguousarray(ego0_loc),
                "g0_in": lg[c]["g0"],
                "rel_in": lg[c]["rel2d"],
                "val_in": lg[c]["val2d"],
                "l2c_in": lg[c]["l2c2d"],
            }
        )

    res = run_bass_kernel_spmd(nc, in_maps, core_ids=list(range(NCORES)))
    LAST_RESULTS = res

    u_g = np.concatenate([res.results[c]["u_out"] for c in range(NCORES)], axis=0)
    i_g = np.concatenate([res.results[c]["i_out"] for c in range(NCORES)], axis=0)
    return u_g, i_g
